# revision 11
# baseline (speedup 1.0000x reference)
"""Trainium2 Bass kernel: segment_sum of edge features into nodes (GNN
aggregation).

out[n, :] = sum of edges[e, :] over edges with receivers[e] == n, for
n in [0, 100000), edges [1000000, 64] fp32 — distributed over 8 NeuronCores.
Cores are value-sharded by receiver range (12500 nodes each, disjoint), so no
cross-core reduction is needed; the host concatenates the shards.

Device algorithm ("identity-matmul tower fold", fp16 end to end):
  - Edge features ride as plain fp16 (end-to-end error ~5e-4 relative vs the
    2e-2 gate), halving input traffic vs an fp32-exact hi+lo split.
  - Host splits nodes with degree > 16 into pseudo-nodes of <= 16 edges,
    sorts pseudo-nodes by degree (desc), and packs 128 per block: pseudo-node
    j of block b puts its k-th edge row at tokens[j, c0[b] + k, :].  A block
    occupies K_b = max-degree-in-block consecutive chunks ("towers"); padding
    is only (K_b - deg) zero slots per node — ~1.5% total, since degrees
    within a sorted block are nearly equal.
  - One matmul folds up to 8 chunks of a tower: lhsT = identity (fp16), rhs =
    tok[:, c:c+K, :], and the out access pattern [[part 128], [0, K], [1, 64]]
    revisits the same 64 PSUM columns for every chunk — PSUM's per-element
    has_written accumulate sums the K chunks in hardware.  (Matmul out free
    iteration is ISA-capped at 512 elements, hence K <= 8 per instruction.)
  - Blocks with K <= 8 use one matmul into one 64-col PSUM slice; blocks with
    K in 9..16 use two independent start=stop=True matmuls into TWO slices,
    which a VectorE tensor_add fuses (with the f32->f16 cast) while staging.
    All matmuls are single-instruction accumulation groups, so no
    cross-instruction has_written hazards exist regardless of scheduling.
  - PSUM slices pack 8 per bank; normal-block slices are staged by ScalarE
    copies (batched over contiguous runs).  Inputs stream on the Sync HWDGE
    ring in ~2 MB slabs; outputs ride the Scalar ring.  Output is exactly one
    64-col fp16 row per pseudo-node (~1.7 MB/core).
  - Host adds pseudo-node rows back into node rows (np.add.at over ~13k rows)
    in f32.
  - Block heights K_b are measured from the actual data (elementwise max
    across the 8 cores' sorted degree profiles) and baked into the compiled
    program inside kernel(); all cores share one SPMD schedule.
"""

import os

import numpy as np

N_EDGES = 1_000_000
N_NODES = 100_000
N_FEAT = 64
N_CORES = 8
NPC = N_NODES // N_CORES  # 12500 nodes per core
K_CAP = 16  # max tower height = max edges per pseudo-node
K_MM = 8  # max chunks foldable by one matmul (512-elem out iteration cap)
BLK = 128  # pseudo-nodes per block (one partition each)
SGRP = 8  # output blocks per stage tile / out DMA
CHUNK_BUDGET = 128  # chunks per input DMA slab (16 KB/partition, ~2.1 MB)

_NC_CACHE = {}
LAST_RESULT = None


def _excl_cumsum(a):
    s = np.zeros_like(a)
    np.cumsum(a[:-1], out=s[1:])
    return s


def _input_groups(k_sched, c0):
    """Greedy consecutive-block slabs of ~CHUNK_BUDGET chunks; the final slab
    is split so the pipeline tail after the last DMA stays short."""
    nb = len(k_sched)
    groups = []
    b = 0
    while b < nb:
        e = b + 1
        while e < nb and c0[e + 1] - c0[b] <= CHUNK_BUDGET:
            e += 1
        groups.append([b, e])
        b = e
    if len(groups) > 1 and c0[groups[-1][1]] - c0[groups[-1][0]] > 32:
        b0, b1 = groups.pop()
        mid = (b0 + b1 + 1) // 2
        groups.extend([[b0, mid], [mid, b1]])
    return groups


def _build_nc(k_sched):
    """Compile the SPMD program for a static tuple of block heights."""
    if k_sched in _NC_CACHE:
        return _NC_CACHE[k_sched]

    import concourse.bass as bass
    import concourse.tile as tile
    from concourse import bacc, mybir

    F16 = mybir.dt.float16
    F32 = mybir.dt.float32

    nb = len(k_sched)
    c0 = np.concatenate([[0], np.cumsum(k_sched)]).astype(np.int64)
    c_total = int(c0[-1])
    igroups = _input_groups(k_sched, c0)
    gmax = max(int(c0[b1] - c0[b0]) for b0, b1 in igroups)
    istart = {b0: (int(c0[b0]), int(c0[b1] - c0[b0])) for b0, b1 in igroups}

    nc = bacc.Bacc("TRN2", target_bir_lowering=False)
    tokens = nc.dram_tensor("tokens", [128, c_total, 64], F16, kind="ExternalInput")
    eye = nc.dram_tensor("eye", [128, 128], F16, kind="ExternalInput")
    out = nc.dram_tensor("out", [128, nb, 64], F16, kind="ExternalOutput")

    with tile.TileContext(nc) as tc:
        with (
            nc.allow_low_precision(reason="fp16 staging is intentional"),
            tc.tile_pool(name="const", bufs=1) as const,
            tc.tile_pool(name="tok", bufs=4) as tokp,
            tc.tile_pool(name="ps", bufs=4, space="PSUM") as psp,
            tc.tile_pool(name="stage", bufs=3) as stp,
        ):
            eye_t = const.tile([128, 128], F16)
            nc.scalar.dma_start(eye_t[:], eye[:])
            # Load the (never-changing) identity into the PE array once; every
            # matmul below sets ldweights=False so walrus emits no per-matmul
            # LDWEIGHTS (~107ns each, ~18us total otherwise).
            nc.tensor.ldweights(eye_t[:])

            tok = None
            tok_c0 = 0
            for b in range(nb):
                if b in istart:
                    gc0, gcn = istart[b]
                    tok = tokp.tile([128, gmax, 64], F16, tag="tok")
                    nc.sync.dma_start(tok[:, 0:gcn, :], tokens[:, gc0 : gc0 + gcn, :])
                    tok_c0 = gc0
                s = b % SGRP
                if s == 0:
                    g0 = b
                    stage = stp.tile([128, SGRP * 64], F16, tag="stage")
                    ps_tiles = [psp.tile([128, 512], F32, tag="ps", name="ps0")]
                    nslice = 0
                    runs = []  # (psum tile idx, slice-in-bank, stage col, nslots)

                def pslice_ap(sl, kk):
                    if sl >= 8 * len(ps_tiles):
                        ps_tiles.append(
                            psp.tile([128, 512], F32, tag="ps", name="ps1")
                        )
                    pstile = ps_tiles[sl // 8]
                    p = pstile[:, (sl % 8) * 64 : (sl % 8 + 1) * 64]
                    return (
                        pstile,
                        p,
                        bass.AP(p.tensor, p.offset, [list(p.ap[0]), [0, kk], [1, 64]]),
                    )

                k = k_sched[b]
                cs = int(c0[b]) - tok_c0
                if k > K_MM:
                    if nslice % 8 == 7:  # keep the pair inside one bank
                        nslice += 1
                    h = (k + 1) // 2
                    _, pA, oA = pslice_ap(nslice, h)
                    _, pB, oB = pslice_ap(nslice + 1, k - h)
                    nc.tensor.matmul(
                        out=oA, lhsT=eye_t[:], rhs=tok[:, cs : cs + h, :],
                        start=True, stop=True,
                    ).ins.ldweights = False
                    nc.tensor.matmul(
                        out=oB, lhsT=eye_t[:], rhs=tok[:, cs + h : cs + k, :],
                        start=True, stop=True,
                    ).ins.ldweights = False
                    # Fold the two 64-col partials with one DVE reduce over the
                    # pair axis (single PSUM operand; f32+f32 -> f16 on write).
                    ti, sl = nslice // 8, nslice % 8
                    pp = ps_tiles[ti][:, sl * 64 : (sl + 2) * 64]
                    pair = bass.AP(
                        pp.tensor, pp.offset, [list(pp.ap[0]), [1, 64], [64, 2]]
                    )
                    nc.vector.tensor_reduce(
                        stage[:, s * 64 : (s + 1) * 64],
                        pair,
                        axis=mybir.AxisListType.X,
                        op=mybir.AluOpType.add,
                    )
                    nslice += 2
                else:
                    _, _, oA = pslice_ap(nslice, k)
                    nc.tensor.matmul(
                        out=oA, lhsT=eye_t[:], rhs=tok[:, cs : cs + k, :],
                        start=True, stop=True,
                    ).ins.ldweights = False
                    ti, sl = nslice // 8, nslice % 8
                    if runs and runs[-1][0] == ti and runs[-1][1] + runs[-1][3] == sl:
                        runs[-1][3] += 1
                    else:
                        runs.append([ti, sl, s, 1])
                    nslice += 1

                if s == SGRP - 1 or b == nb - 1:
                    for ti, sl, sc, n in runs:
                        nc.scalar.copy(
                            stage[:, sc * 64 : (sc + n) * 64],
                            ps_tiles[ti][:, sl * 64 : (sl + n) * 64],
                        )
                    nblk = s + 1
                    nc.scalar.dma_start(
                        out[:, g0 : g0 + nblk, :], stage[:, 0 : nblk * 64]
                    )
    nc.compile()
    _NC_CACHE[k_sched] = nc
    return nc


def _numpy_segment_sum(edges, receivers, n_nodes):
    out = np.zeros((n_nodes, edges.shape[1]), np.float32)
    r = np.asarray(receivers).astype(np.int64)
    ok = (r >= 0) & (r < n_nodes)
    np.add.at(out, r[ok], np.asarray(edges, np.float32)[ok])
    return out


def kernel(edges, nodes, receivers):
    global LAST_RESULT

    edges = np.ascontiguousarray(edges, dtype=np.float32)
    n_nodes = nodes.shape[0]
    r = np.asarray(receivers).astype(np.int64)
    if (
        edges.shape != (N_EDGES, N_FEAT)
        or n_nodes != N_NODES
        or r.shape != (N_EDGES,)
        or ((r < 0) | (r >= N_NODES)).any()
        or os.environ.get("KERNEL_FORCE_NUMPY")
    ):
        return _numpy_segment_sum(edges, receivers, n_nodes)

    order = np.argsort(r, kind="stable")
    r_s = r[order]
    bounds = np.searchsorted(r_s, NPC * np.arange(N_CORES + 1))
    hi_all = edges.astype(np.float16)

    # ---- pass 1: per-core pseudo-node construction + sorted degree profiles
    per_core = []
    nb_max = 0
    for i in range(N_CORES):
        lo_b, hi_b = bounds[i], bounds[i + 1]
        idx = order[lo_b:hi_b]
        rr = (r_s[lo_b:hi_b] - NPC * i).astype(np.int64)
        d = np.bincount(rr, minlength=NPC)
        n_parts = np.maximum((d + K_CAP - 1) // K_CAP, 1)
        pseudo_base = _excl_cumsum(n_parts)
        n_pseudo = int(n_parts.sum())
        pseudo_orig = np.repeat(np.arange(NPC), n_parts)
        part_idx = np.arange(n_pseudo) - pseudo_base[pseudo_orig]
        pseudo_deg = np.minimum(d[pseudo_orig] - K_CAP * part_idx, K_CAP)
        sort_ord = np.argsort(-pseudo_deg, kind="stable")
        inv = np.empty(n_pseudo, np.int64)
        inv[sort_ord] = np.arange(n_pseudo)
        deg_sorted = pseudo_deg[sort_ord]
        per_core.append(
            (idx, rr, d, pseudo_base, inv, pseudo_orig, sort_ord, n_pseudo, deg_sorted)
        )
        nb_max = max(nb_max, (n_pseudo + BLK - 1) // BLK)

    # Static schedule: per-block height = max over cores of block max degree.
    k_all = np.zeros((N_CORES, nb_max), np.int64)
    for i in range(N_CORES):
        deg_sorted = per_core[i][8]
        nb_i = (len(deg_sorted) + BLK - 1) // BLK
        k_all[i, :nb_i] = deg_sorted[0 : nb_i * BLK : BLK]
    k_sched_arr = k_all.max(axis=0)
    nb = int(np.max(np.nonzero(k_sched_arr)[0])) + 1 if k_sched_arr.any() else 0
    if nb == 0:
        return np.zeros((N_NODES, N_FEAT), np.float32)
    k_sched = tuple(int(x) for x in k_sched_arr[:nb])
    c0 = np.concatenate([[0], np.cumsum(k_sched)]).astype(np.int64)
    c_total = int(c0[-1])

    nc = _build_nc(k_sched)

    # ---- pass 2: scatter edges into per-core token arrays
    ar = np.arange(128)
    eye_np = np.zeros((128, 128), np.float16)
    eye_np[ar, ar] = 1.0
    in_maps = []
    for i in range(N_CORES):
        idx, rr, d, pseudo_base, inv, _, _, _, _ = per_core[i]
        node_first = _excl_cumsum(d)
        rank = np.arange(len(rr)) - node_first[rr]
        pn = pseudo_base[rr] + rank // K_CAP
        rk = rank % K_CAP
        q = inv[pn]
        blk = q >> 7
        j = q & 127
        chunk = c0[blk] + rk
        tokens = np.zeros((128, c_total, 64), np.float16)
        tokens[j, chunk, :] = hi_all[idx]
        in_maps.append({"tokens": tokens, "eye": eye_np})

    from concourse.bass_utils import run_bass_kernel_spmd

    res = run_bass_kernel_spmd(nc, in_maps, core_ids=list(range(N_CORES)))
    LAST_RESULT = res

    # ---- unshard: row q of dev out is pseudo-node sort_ord[q]'s sum.
    full = np.zeros((N_NODES, N_FEAT), np.float32)
    for i in range(N_CORES):
        dev = res.results[i]["out"]  # [128, nb, 64] f16
        rows = dev.transpose(1, 0, 2).reshape(-1, 64).astype(np.float32)
        _, _, _, _, _, pseudo_orig, sort_ord, n_pseudo, _ = per_core[i]
        m = min(n_pseudo, nb * BLK)  # trailing deg-0 pseudo-nodes may be trimmed
        block = full[i * NPC : (i + 1) * NPC]
        np.add.at(block, pseudo_orig[sort_ord[:m]], rows[:m])

    return full


# revision 12
# speedup vs baseline: 1.0109x; 1.0109x over previous
"""Trainium2 Bass kernel: segment_sum of edge features into nodes (GNN
aggregation).

out[n, :] = sum of edges[e, :] over edges with receivers[e] == n, for
n in [0, 100000), edges [1000000, 64] fp32 — distributed over 8 NeuronCores.
Cores are value-sharded by receiver range (12500 nodes each, disjoint), so no
cross-core reduction is needed; the host concatenates the shards.

Device algorithm ("block-ones matmul tower fold", fp16 end to end):
  - Edge features ride as plain fp16 (end-to-end error ~5e-4 relative vs the
    2e-2 gate), halving input traffic vs an fp32-exact hi+lo split.
  - Host splits nodes with degree > 16 into pseudo-nodes of <= 16 edges,
    sorts pseudo-nodes by degree (desc), and packs 64 per block, 2 slots per
    node per chunk: pseudo-node j of block b puts its e-th edge row at
    tokens[2j + (e&1), c0[b] + (e>>1), :].  A block occupies
    K_b = ceil(max-degree-in-block / 2) <= 8 consecutive chunks ("towers");
    padding is ~3% (odd-degree slots + within-block degree spread).
  - ONE matmul per block folds the whole tower: lhsT = static block-ones
    [128, 64] (ones2[s, m] = 1 iff s//2 == m, so out row m sums slots 2m and
    2m+1), rhs = tok[:, c:c+K, :], and the out access pattern
    [[64 part], [0, K], [1, 64]] revisits the same 64 PSUM columns for every
    chunk — PSUM's per-element has_written accumulate sums the K chunks in
    hardware.  The 64-wide lhsT halves the per-matmul LDWEIGHTS cost (53 ns)
    vs a 128-wide identity, keeping the PE comfortably under the DMA stream.
  - Matmul out free iteration is ISA-capped at 512 elements, hence K <= 8 per
    instruction — guaranteed here since pseudo-degree <= 16.
  - Two blocks stack per 128-partition group (tile_position column tiling at
    partition 0/64); 16 blocks fill one 2KB PSUM bank; one ScalarE/VectorE
    copy (alternating) casts the bank to fp16 in SBUF.  Inputs stream on the
    Sync HWDGE ring in ~2 MB slabs; outputs ride the Scalar ring.  Output is
    exactly one 64-col fp16 row per pseudo-node (~1.7 MB/core).
  - Host adds pseudo-node rows back into node rows (np.add.at over ~13k rows)
    in f32.
  - Block heights K_b are measured from the actual data (elementwise max
    across the 8 cores' sorted degree profiles) and baked into the compiled
    program inside kernel(); all cores share one SPMD schedule.
"""

import os

import numpy as np

N_EDGES = 1_000_000
N_NODES = 100_000
N_FEAT = 64
N_CORES = 8
NPC = N_NODES // N_CORES  # 12500 nodes per core
K_CAP = 16  # max edges per pseudo-node -> tower height ceil(16/2) = 8 chunks
BLK = 64  # pseudo-nodes per block (two slots each)
BPB = 16  # blocks per PSUM bank (2 partition groups x 8 column slices)
CHUNK_BUDGET = 128  # chunks per input DMA slab (16 KB/partition, ~2.1 MB)

_NC_CACHE = {}
LAST_RESULT = None


def _excl_cumsum(a):
    s = np.zeros_like(a)
    np.cumsum(a[:-1], out=s[1:])
    return s


def _input_groups(k_sched, c0):
    """Greedy consecutive-block slabs of ~CHUNK_BUDGET chunks; the final slab
    is split so the pipeline tail after the last DMA stays short."""
    nb = len(k_sched)
    groups = []
    b = 0
    while b < nb:
        e = b + 1
        while e < nb and c0[e + 1] - c0[b] <= CHUNK_BUDGET:
            e += 1
        groups.append([b, e])
        b = e
    if len(groups) > 1 and c0[groups[-1][1]] - c0[groups[-1][0]] > 32:
        b0, b1 = groups.pop()
        mid = (b0 + b1 + 1) // 2
        groups.extend([[b0, mid], [mid, b1]])
    return groups


def _build_nc(k_sched):
    """Compile the SPMD program for a static tuple of block heights."""
    if k_sched in _NC_CACHE:
        return _NC_CACHE[k_sched]

    import concourse.bass as bass
    import concourse.tile as tile
    from concourse import bacc, mybir

    F16 = mybir.dt.float16
    F32 = mybir.dt.float32

    nb = len(k_sched)
    nbd = (nb + 1) // 2  # dram col-blocks (2 blocks stack per 128 partitions)
    c0 = np.concatenate([[0], np.cumsum(k_sched)]).astype(np.int64)
    c_total = int(c0[-1])
    igroups = _input_groups(k_sched, c0)
    gmax = max(int(c0[b1] - c0[b0]) for b0, b1 in igroups)
    istart = {b0: (int(c0[b0]), int(c0[b1] - c0[b0])) for b0, b1 in igroups}

    nc = bacc.Bacc("TRN2", target_bir_lowering=False)
    tokens = nc.dram_tensor("tokens", [128, c_total, 64], F16, kind="ExternalInput")
    ones2 = nc.dram_tensor("ones2", [128, 64], F16, kind="ExternalInput")
    out = nc.dram_tensor("out", [128, nbd, 64], F16, kind="ExternalOutput")

    with tile.TileContext(nc) as tc:
        with (
            nc.allow_low_precision(reason="fp16 staging is intentional"),
            tc.tile_pool(name="const", bufs=1) as const,
            tc.tile_pool(name="tok", bufs=4) as tokp,
            tc.tile_pool(name="ps", bufs=4, space="PSUM") as psp,
            tc.tile_pool(name="stage", bufs=3) as stp,
        ):
            ones2_t = const.tile([128, 64], F16)
            nc.scalar.dma_start(ones2_t[:], ones2[:])

            tok = None
            tok_c0 = 0
            tick = 0
            for b in range(nb):
                if b in istart:
                    gc0, gcn = istart[b]
                    tok = tokp.tile([128, gmax, 64], F16, tag="tok")
                    nc.sync.dma_start(tok[:, 0:gcn, :], tokens[:, gc0 : gc0 + gcn, :])
                    tok_c0 = gc0
                w = b % BPB  # position within the PSUM bank
                if w == 0:
                    g0 = b
                    ps = psp.tile([128, 512], F32, tag="ps")
                k = k_sched[b]
                assert 0 < k <= 8
                cs = int(c0[b]) - tok_c0
                prow = 64 * (w % 2)
                slot = w // 2
                pslice = ps[prow : prow + 64, slot * 64 : (slot + 1) * 64]
                o = bass.AP(
                    pslice.tensor,
                    pslice.offset,
                    [list(pslice.ap[0]), [0, k], [1, 64]],
                )
                nc.tensor.matmul(
                    out=o,
                    lhsT=ones2_t[:],
                    rhs=tok[:, cs : cs + k, :],
                    start=True,
                    stop=True,
                )
                if w == BPB - 1 or b == nb - 1:
                    ncols = (w // 2 + 1) * 64
                    stage = stp.tile([128, 512], F16, tag="stage")
                    if tick % 2:
                        nc.vector.tensor_copy(stage[:, 0:ncols], ps[:, 0:ncols])
                    else:
                        nc.scalar.copy(stage[:, 0:ncols], ps[:, 0:ncols])
                    tick += 1
                    d0 = g0 // 2
                    nc.scalar.dma_start(
                        out[:, d0 : d0 + ncols // 64, :], stage[:, 0:ncols]
                    )
    nc.compile()
    _NC_CACHE[k_sched] = nc
    return nc


def _numpy_segment_sum(edges, receivers, n_nodes):
    out = np.zeros((n_nodes, edges.shape[1]), np.float32)
    r = np.asarray(receivers).astype(np.int64)
    ok = (r >= 0) & (r < n_nodes)
    np.add.at(out, r[ok], np.asarray(edges, np.float32)[ok])
    return out


def kernel(edges, nodes, receivers):
    global LAST_RESULT

    edges = np.ascontiguousarray(edges, dtype=np.float32)
    n_nodes = nodes.shape[0]
    r = np.asarray(receivers).astype(np.int64)
    if (
        edges.shape != (N_EDGES, N_FEAT)
        or n_nodes != N_NODES
        or r.shape != (N_EDGES,)
        or ((r < 0) | (r >= N_NODES)).any()
        or os.environ.get("KERNEL_FORCE_NUMPY")
    ):
        return _numpy_segment_sum(edges, receivers, n_nodes)

    order = np.argsort(r, kind="stable")
    r_s = r[order]
    bounds = np.searchsorted(r_s, NPC * np.arange(N_CORES + 1))
    hi_all = edges.astype(np.float16)

    # ---- pass 1: per-core pseudo-node construction + sorted degree profiles
    per_core = []
    nb_max = 0
    for i in range(N_CORES):
        lo_b, hi_b = bounds[i], bounds[i + 1]
        idx = order[lo_b:hi_b]
        rr = (r_s[lo_b:hi_b] - NPC * i).astype(np.int64)
        d = np.bincount(rr, minlength=NPC)
        n_parts = np.maximum((d + K_CAP - 1) // K_CAP, 1)
        pseudo_base = _excl_cumsum(n_parts)
        n_pseudo = int(n_parts.sum())
        pseudo_orig = np.repeat(np.arange(NPC), n_parts)
        part_idx = np.arange(n_pseudo) - pseudo_base[pseudo_orig]
        pseudo_deg = np.minimum(d[pseudo_orig] - K_CAP * part_idx, K_CAP)
        sort_ord = np.argsort(-pseudo_deg, kind="stable")
        inv = np.empty(n_pseudo, np.int64)
        inv[sort_ord] = np.arange(n_pseudo)
        deg_sorted = pseudo_deg[sort_ord]
        per_core.append(
            (idx, rr, d, pseudo_base, inv, pseudo_orig, sort_ord, n_pseudo, deg_sorted)
        )
        nb_max = max(nb_max, (n_pseudo + BLK - 1) // BLK)

    # Static schedule: per-block tower height = ceil(block max degree / 2),
    # maxed over cores.
    k_all = np.zeros((N_CORES, nb_max), np.int64)
    for i in range(N_CORES):
        deg_sorted = per_core[i][8]
        nb_i = (len(deg_sorted) + BLK - 1) // BLK
        k_all[i, :nb_i] = (deg_sorted[0 : nb_i * BLK : BLK] + 1) // 2
    k_sched_arr = k_all.max(axis=0)
    nb = int(np.max(np.nonzero(k_sched_arr)[0])) + 1 if k_sched_arr.any() else 0
    if nb == 0:
        return np.zeros((N_NODES, N_FEAT), np.float32)
    k_sched = tuple(int(x) for x in k_sched_arr[:nb])
    c0 = np.concatenate([[0], np.cumsum(k_sched)]).astype(np.int64)
    c_total = int(c0[-1])

    nc = _build_nc(k_sched)

    # ---- pass 2: scatter edges into per-core token arrays
    ones2_np = np.zeros((128, 64), np.float16)
    ones2_np[np.arange(128), np.arange(128) // 2] = 1.0
    in_maps = []
    for i in range(N_CORES):
        idx, rr, d, pseudo_base, inv, _, _, _, _ = per_core[i]
        node_first = _excl_cumsum(d)
        rank = np.arange(len(rr)) - node_first[rr]
        pn = pseudo_base[rr] + rank // K_CAP
        rk = rank % K_CAP
        q = inv[pn]
        blk = q // BLK
        j = q % BLK
        part = 2 * j + (rk & 1)
        chunk = c0[blk] + (rk >> 1)
        tokens = np.zeros((128, c_total, 64), np.float16)
        tokens[part, chunk, :] = hi_all[idx]
        in_maps.append({"tokens": tokens, "ones2": ones2_np})

    from concourse.bass_utils import run_bass_kernel_spmd

    res = run_bass_kernel_spmd(nc, in_maps, core_ids=list(range(N_CORES)))
    LAST_RESULT = res

    # ---- unshard: pseudo-node sort_ord[q]'s sum lives at
    # dev[64*(blk&1) + j, blk>>1, :] with blk = q//64, j = q%64.
    full = np.zeros((N_NODES, N_FEAT), np.float32)
    for i in range(N_CORES):
        dev = res.results[i]["out"]  # [128, nbd, 64] f16
        _, _, _, _, _, pseudo_orig, sort_ord, n_pseudo, _ = per_core[i]
        m = min(n_pseudo, nb * BLK)  # trailing deg-0 pseudo-nodes may be trimmed
        q = np.arange(m)
        blk = q // BLK
        j = q % BLK
        vals = dev[64 * (blk & 1) + j, blk >> 1, :].astype(np.float32)
        block = full[i * NPC : (i + 1) * NPC]
        np.add.at(block, pseudo_orig[sort_ord[:m]], vals)

    return full
